# revision 26
# baseline (speedup 1.0000x reference)
"""LoRA Linear kernel for 8x TRN2 NeuronCores (Bass/Tile).

Computes  y = x @ W^T + b + 2.0 * ((x @ A^T) @ B^T)   for
  x [4, 2048, 4096] f32, W [4096, 4096], b [4096], A [16, 4096], B [4096, 16].

Strategy (v7):
  - Algebraic fold: with dropout=0 the LoRA path is linear, so
    W_eff = W + 2.0 * (B @ A) merged on the host (rank-16 update, 0.1% of
    the problem FLOPs).  The device runs a single dense matmul + bias.
  - Data-parallel over tokens: 8192 tokens -> 1024 per core.
  - Stationary operand is the W o-tile [128d, 128o]; the moving operand
    sweeps tokens, so one Ldweights serves the two 512-token chunks
    (redundant Ldweights are deduped post-tile; they cost ~46ns each).
  - Output computed as [O, TC] per core (o on partitions); host transposes.
    That layout makes the bias a per-partition constant, added for free
    during the PSUM->SBUF drain (ACT activation bias / DVE tensor_scalar).
  - x arrives in ds-major 1MB slices on both HWDGE queues so the first
    o-tile's accumulation chain can start ~12us in, tracking arrivals.
  - W is host-prepacked per o-tile so each DMA line is 8KB contiguous.
"""

import os

import numpy as np
import ml_dtypes

_BF16 = ml_dtypes.bfloat16

# Problem constants (hardcoded per harness contract).
_B, _S, _D, _O, _R = 4, 2048, 4096, 4096, 16
_T = _B * _S          # 8192 tokens
_NCORES = 8
_TC = _T // _NCORES   # 1024 tokens per core

P = 128
DS = _D // P          # 32 contraction subtiles
NOT = _O // P         # 32 o-tiles
TCH = 512             # token chunk (moving N)
NCH = _TC // TCH      # 2 chunks per core

_cache = {}

# Set by kernel() when KERNEL_TRACE=1; read by test.py for exec_time_ns.
LAST_RESULT = None


def _build_module():
    import concourse.bass as bass
    import concourse.bacc as bacc
    import concourse.mybir as mybir
    import concourse.tile as tile
    from concourse.bass import ts

    bf16 = mybir.dt.bfloat16
    f32 = mybir.dt.float32

    nc = bacc.Bacc("TRN2", target_bir_lowering=False, debug=False)
    xp_d = nc.dram_tensor("xp", [P, DS, _TC], bf16, kind="ExternalInput")
    Wp_d = nc.dram_tensor("Wp", [NOT * P, DS, P], bf16, kind="ExternalInput")
    bvec_d = nc.dram_tensor("bvec", [P, NOT], f32, kind="ExternalInput")
    out_d = nc.dram_tensor("out", [_O, _TC], f32, kind="ExternalOutput")

    with tile.TileContext(nc) as tc:
        with (
            tc.tile_pool(name="const", bufs=1) as cpool,
            tc.tile_pool(name="wpool", bufs=6) as wpool,
            tc.tile_pool(name="opool", bufs=3) as opool,
            tc.tile_pool(name="ps_mm", bufs=3, space="PSUM") as ps_pool,
        ):
            NG = 8           # ds-group x tiles; each holds both token chunks
            GD = DS // NG    # 4 ds per group
            xg = [
                cpool.tile([P, GD, _TC], bf16, name=f"xg{g}") for g in range(NG)
            ]
            b_sb = cpool.tile([P, NOT], f32)

            def xmov(ds, c):
                return xg[ds // GD][:, ds % GD, ts(c, TCH)]

            # x in ds-major 1MB slices alternating across both HWDGE queues;
            # the first two W tiles interleave with the early x slices so
            # the o-tile chains can start ~10us in.
            nc.sync.dma_start(b_sb[:], bvec_d[:, :])
            W01 = [
                cpool.tile([P, DS, P], bf16, name=f"W0{ot}") for ot in range(2)
            ]
            nc.sync.dma_start(xg[0][:], xp_d[:, 0:GD, :])
            nc.scalar.dma_start(xg[1][:], xp_d[:, GD : 2 * GD, :])
            # First W tile split across both queues so the first chain can
            # start as early as possible; W1 right behind.
            nc.sync.dma_start(W01[0][:, 0 : DS // 2, :], Wp_d[0:P, 0 : DS // 2, :])
            nc.scalar.dma_start(
                W01[0][:, DS // 2 : DS, :], Wp_d[0:P, DS // 2 : DS, :]
            )
            nc.sync.dma_start(W01[1][:, 0 : DS // 2, :], Wp_d[P : 2 * P, 0 : DS // 2, :])
            nc.scalar.dma_start(
                W01[1][:, DS // 2 : DS, :], Wp_d[P : 2 * P, DS // 2 : DS, :]
            )
            for g in range(2, NG):
                q = nc.sync if g % 2 == 0 else nc.scalar
                q.dma_start(xg[g][:], xp_d[:, g * GD : (g + 1) * GD, :])

            def drain(ot, ps):
                # Drain PSUM -> SBUF with the bias folded in (per-partition
                # constant in this [o, t] layout); ACT and DVE work
                # different banks in parallel; out DMAs alternate queues.
                QW = TCH // 2
                bias = b_sb[:, ot : ot + 1]
                for c in range(NCH):
                    for h in range(2):
                        q = c * 2 + h
                        qt = opool.tile([P, QW], f32, name=f"ot_q{q}")
                        if h == 0:
                            nc.scalar.add(qt[:], ps[c][:, ts(h, QW)], bias)
                        else:
                            nc.vector.tensor_scalar_add(
                                qt[:], ps[c][:, ts(h, QW)], bias
                            )
                        dq = nc.sync if q % 2 == 0 else nc.scalar
                        dq.dma_start(out_d[ts(ot, P), q * QW : (q + 1) * QW], qt[:])

            # Phase 1: o-tiles 0 and 1 interleaved by ds-group, consuming
            # each arriving x slice twice so the PE keeps pace with the
            # x DMA stream instead of stalling on it.
            ps01 = [
                [ps_pool.tile([P, TCH], f32, name=f"ps{c}") for c in range(NCH)]
                for _ in range(2)
            ]
            for g in range(NG):
                for ot in range(2):
                    for dsl in range(GD):
                        ds = g * GD + dsl
                        for c in range(NCH):
                            nc.tensor.matmul(
                                ps01[ot][c][:],
                                W01[ot][:, ds, :],
                                xmov(ds, c),
                                start=(ds == 0),
                                stop=(ds == DS - 1),
                            )
            for ot in range(2):
                drain(ot, ps01[ot])

            # Phase 2: remaining o-tiles, x fully resident, pure stream.
            for ot in range(2, NOT):
                Wt = wpool.tile([P, DS, P], bf16)
                nc.scalar.dma_start(Wt[:], Wp_d[ts(ot, P), :, :])
                ps = [
                    ps_pool.tile([P, TCH], f32, name=f"ps{c}") for c in range(NCH)
                ]
                for ds in range(DS):
                    for c in range(NCH):
                        nc.tensor.matmul(
                            ps[c][:],
                            Wt[:, ds, :],
                            xmov(ds, c),
                            start=(ds == 0),
                            stop=(ds == DS - 1),
                        )
                drain(ot, ps)

    _dedup_ldweights(nc, mybir)
    nc.compile()
    return nc


def _dedup_ldweights(nc, mybir):
    """Drop PE Ldweights that reload the stationary already in the array.

    The tile pass lowers every matmul to an Ldweights+Matmult pair even when
    consecutive matmuls share the stationary operand.  The redundant reload
    costs PE cycles (~46ns exposed per pair at N=512).  Weights persist in
    the array across Matmults, so a back-to-back identical Ldweights with no
    semaphore activity is dead.
    """
    n_drop = 0
    for fn in nc.m.functions:
        for blk in fn.blocks:
            insts = blk.instructions
            new = []
            prev_key = None
            for inst in insts:
                if inst.engine != mybir.EngineType.PE:
                    new.append(inst)
                    continue
                if isinstance(inst, mybir.InstLdweights):
                    key = str(inst.ins[0])
                    if (
                        key == prev_key
                        and not inst.has_wait()
                        and not inst.has_update()
                    ):
                        n_drop += 1
                        continue
                    prev_key = key
                elif isinstance(inst, mybir.InstMatmult):
                    if inst.is_transpose:
                        prev_key = None
                elif isinstance(inst, mybir.InstEventSemaphore):
                    pass
                else:
                    prev_key = None
                new.append(inst)
            blk.instructions = new
    if os.environ.get("KERNEL_DEBUG"):
        print(f"_dedup_ldweights: dropped {n_drop}")


def kernel(x, W, b, lora_A, lora_B):
    global LAST_RESULT
    from concourse.bass_utils import run_bass_kernel_spmd

    if "nc" not in _cache:
        _cache["nc"] = _build_module()
    nc = _cache["nc"]

    # Fold the LoRA rank-16 update into W (dropout=0 makes it exact):
    # y = x @ (W + 2 B A)^T + b
    Weff = W.astype(np.float64) + 2.0 * (
        lora_B.astype(np.float64) @ lora_A.astype(np.float64)
    )

    xf = np.ascontiguousarray(x.reshape(_T, _D)).astype(_BF16)
    xT = np.ascontiguousarray(xf.T)                              # [D, T]
    # [D, T] -> [p, ds, T] so each DMA line is contiguous per partition
    xprep = np.ascontiguousarray(xT.reshape(DS, P, _T).transpose(1, 0, 2))
    WT = Weff.astype(_BF16).T                                    # [D, O]
    # [ds, p, ot, o] -> [ot, p, ds, o] -> [ot*p, ds, o]: 8KB contiguous lines
    Wprep = np.ascontiguousarray(
        WT.reshape(DS, P, NOT, P).transpose(2, 1, 0, 3)
    ).reshape(NOT * P, DS, P)
    bprep = np.ascontiguousarray(b.astype(np.float32).reshape(NOT, P).T)

    in_maps = []
    for c in range(_NCORES):
        t0 = c * _TC
        in_maps.append(
            {
                "xp": np.ascontiguousarray(xprep[:, :, t0 : t0 + _TC]),
                "Wp": Wprep,
                "bvec": bprep,
            }
        )

    trace = os.environ.get("KERNEL_TRACE", "0") == "1"
    res = run_bass_kernel_spmd(
        nc,
        in_maps,
        core_ids=list(range(_NCORES)),
        trace=trace,
    )
    LAST_RESULT = res

    out = np.empty((_T, _O), dtype=np.float32)
    for c, r in enumerate(res.results):
        out[c * _TC : (c + 1) * _TC, :] = r["out"].T
    return out.reshape(_B, _S, _O)


# revision 27
# speedup vs baseline: 1.0070x; 1.0070x over previous
"""LoRA Linear kernel for 8x TRN2 NeuronCores (Bass/Tile).

Computes  y = x @ W^T + b + 2.0 * ((x @ A^T) @ B^T)   for
  x [4, 2048, 4096] f32, W [4096, 4096], b [4096], A [16, 4096], B [4096, 16].

Strategy (v7):
  - Algebraic fold: with dropout=0 the LoRA path is linear, so
    W_eff = W + 2.0 * (B @ A) merged on the host (rank-16 update, 0.1% of
    the problem FLOPs).  The device runs a single dense matmul + bias.
  - Data-parallel over tokens: 8192 tokens -> 1024 per core.
  - Stationary operand is the W o-tile [128d, 128o]; the moving operand
    sweeps tokens, so one Ldweights serves the two 512-token chunks
    (redundant Ldweights are deduped post-tile; they cost ~46ns each).
  - Output computed as [O, TC] per core (o on partitions); host transposes.
    That layout makes the bias a per-partition constant, added for free
    during the PSUM->SBUF drain (ACT activation bias / DVE tensor_scalar).
  - x arrives in ds-major 1MB slices on both HWDGE queues so the first
    o-tile's accumulation chain can start ~12us in, tracking arrivals.
  - W is host-prepacked per o-tile so each DMA line is 8KB contiguous.
"""

import os

import numpy as np
import ml_dtypes

_BF16 = ml_dtypes.bfloat16

# Problem constants (hardcoded per harness contract).
_B, _S, _D, _O, _R = 4, 2048, 4096, 4096, 16
_T = _B * _S          # 8192 tokens
_NCORES = 8
_TC = _T // _NCORES   # 1024 tokens per core

P = 128
DS = _D // P          # 32 contraction subtiles
NOT = _O // P         # 32 o-tiles
TCH = 512             # token chunk (moving N)
NCH = _TC // TCH      # 2 chunks per core

_cache = {}

# Set by kernel() when KERNEL_TRACE=1; read by test.py for exec_time_ns.
LAST_RESULT = None


def _build_module():
    import concourse.bacc as bacc
    import concourse.mybir as mybir
    import concourse.tile as tile
    from concourse.bass import ts

    bf16 = mybir.dt.bfloat16
    f32 = mybir.dt.float32

    nc = bacc.Bacc("TRN2", target_bir_lowering=False, debug=False)
    xp_d = nc.dram_tensor("xp", [P, DS, _TC], bf16, kind="ExternalInput")
    Wp_d = nc.dram_tensor("Wp", [NOT * P, DS, P], bf16, kind="ExternalInput")
    bvec_d = nc.dram_tensor("bvec", [P, NOT], f32, kind="ExternalInput")
    out_d = nc.dram_tensor("out", [_O, _TC], f32, kind="ExternalOutput")

    with tile.TileContext(nc) as tc:
        with (
            tc.tile_pool(name="const", bufs=1) as cpool,
            tc.tile_pool(name="wpool", bufs=6) as wpool,
            tc.tile_pool(name="opool", bufs=3) as opool,
            tc.tile_pool(name="ps_mm", bufs=3, space="PSUM") as ps_pool,
        ):
            NG = 8           # ds-group x tiles; each holds both token chunks
            GD = DS // NG    # 4 ds per group
            xg = [
                cpool.tile([P, GD, _TC], bf16, name=f"xg{g}") for g in range(NG)
            ]
            b_sb = cpool.tile([P, NOT], f32)

            def xmov(ds, c):
                return xg[ds // GD][:, ds % GD, ts(c, TCH)]

            # x in ds-major 1MB slices alternating across both HWDGE queues;
            # the first two W tiles interleave with the early x slices so
            # the o-tile chains can start ~10us in.
            nc.sync.dma_start(b_sb[:], bvec_d[:, :])
            W01 = [
                cpool.tile([P, DS, P], bf16, name=f"W0{ot}") for ot in range(2)
            ]
            nc.sync.dma_start(xg[0][:], xp_d[:, 0:GD, :])
            nc.scalar.dma_start(xg[1][:], xp_d[:, GD : 2 * GD, :])
            # First W tile split across both queues so the first chain can
            # start as early as possible; W1 right behind.
            nc.sync.dma_start(W01[0][:, 0 : DS // 2, :], Wp_d[0:P, 0 : DS // 2, :])
            nc.scalar.dma_start(
                W01[0][:, DS // 2 : DS, :], Wp_d[0:P, DS // 2 : DS, :]
            )
            nc.sync.dma_start(W01[1][:, 0 : DS // 2, :], Wp_d[P : 2 * P, 0 : DS // 2, :])
            nc.scalar.dma_start(
                W01[1][:, DS // 2 : DS, :], Wp_d[P : 2 * P, DS // 2 : DS, :]
            )
            for g in range(2, NG):
                q = nc.sync if g % 2 == 0 else nc.scalar
                q.dma_start(xg[g][:], xp_d[:, g * GD : (g + 1) * GD, :])

            def drain(ot, ps):
                # Drain PSUM -> SBUF with the bias folded in (per-partition
                # constant in this [o, t] layout); ACT and DVE work
                # different banks in parallel; out DMAs alternate queues.
                QW = TCH // 2
                bias = b_sb[:, ot : ot + 1]
                for c in range(NCH):
                    for h in range(2):
                        q = c * 2 + h
                        qt = opool.tile([P, QW], f32, name=f"ot_q{q}")
                        if h == 0:
                            nc.scalar.add(qt[:], ps[c][:, ts(h, QW)], bias)
                        else:
                            nc.vector.tensor_scalar_add(
                                qt[:], ps[c][:, ts(h, QW)], bias
                            )
                        dq = nc.sync if q % 2 == 0 else nc.scalar
                        dq.dma_start(out_d[ts(ot, P), q * QW : (q + 1) * QW], qt[:])

            # Phase 1: o-tiles 0 and 1 interleaved by ds-group, consuming
            # each arriving x slice twice so the PE keeps pace with the
            # x DMA stream instead of stalling on it.
            ps01 = [
                [ps_pool.tile([P, TCH], f32, name=f"ps{c}") for c in range(NCH)]
                for _ in range(2)
            ]
            for g in range(NG):
                for ot in range(2):
                    for dsl in range(GD):
                        ds = g * GD + dsl
                        for c in range(NCH):
                            nc.tensor.matmul(
                                ps01[ot][c][:],
                                W01[ot][:, ds, :],
                                xmov(ds, c),
                                start=(ds == 0),
                                stop=(ds == DS - 1),
                            )
            for ot in range(2):
                drain(ot, ps01[ot])

            # Phase 2: remaining o-tiles, x fully resident, pure stream.
            for ot in range(2, NOT):
                Wt = wpool.tile([P, DS, P], bf16)
                nc.scalar.dma_start(Wt[:], Wp_d[ts(ot, P), :, :])
                ps = [
                    ps_pool.tile([P, TCH], f32, name=f"ps{c}") for c in range(NCH)
                ]
                for ds in range(DS):
                    for c in range(NCH):
                        nc.tensor.matmul(
                            ps[c][:],
                            Wt[:, ds, :],
                            xmov(ds, c),
                            start=(ds == 0),
                            stop=(ds == DS - 1),
                        )
                drain(ot, ps)

    _dedup_ldweights(nc, mybir)
    nc.compile()
    return nc


def _dedup_ldweights(nc, mybir):
    """Drop PE Ldweights that reload the stationary already in the array.

    The tile pass lowers every matmul to an Ldweights+Matmult pair even when
    consecutive matmuls share the stationary operand.  The redundant reload
    costs PE cycles (~46ns exposed per pair at N=512).  Weights persist in
    the array across Matmults, so a back-to-back identical Ldweights with no
    semaphore activity is dead.
    """
    n_drop = 0
    for fn in nc.m.functions:
        for blk in fn.blocks:
            insts = blk.instructions
            new = []
            prev_key = None
            for inst in insts:
                if inst.engine != mybir.EngineType.PE:
                    new.append(inst)
                    continue
                if isinstance(inst, mybir.InstLdweights):
                    key = str(inst.ins[0])
                    if (
                        key == prev_key
                        and not inst.has_wait()
                        and not inst.has_update()
                    ):
                        n_drop += 1
                        continue
                    prev_key = key
                elif isinstance(inst, mybir.InstMatmult):
                    if inst.is_transpose:
                        prev_key = None
                elif isinstance(inst, mybir.InstEventSemaphore):
                    pass
                else:
                    prev_key = None
                new.append(inst)
            blk.instructions = new
    if os.environ.get("KERNEL_DEBUG"):
        print(f"_dedup_ldweights: dropped {n_drop}")


def kernel(x, W, b, lora_A, lora_B):
    global LAST_RESULT
    from concourse.bass_utils import run_bass_kernel_spmd

    if "nc" not in _cache:
        _cache["nc"] = _build_module()
    nc = _cache["nc"]

    # Fold the LoRA rank-16 update into W (dropout=0 makes it exact):
    # y = x @ (W + 2 B A)^T + b
    Weff = W.astype(np.float64) + 2.0 * (
        lora_B.astype(np.float64) @ lora_A.astype(np.float64)
    )

    xf = np.ascontiguousarray(x.reshape(_T, _D)).astype(_BF16)
    xT = np.ascontiguousarray(xf.T)                              # [D, T]
    # [D, T] -> [p, ds, T] so each DMA line is contiguous per partition
    xprep = np.ascontiguousarray(xT.reshape(DS, P, _T).transpose(1, 0, 2))
    WT = Weff.astype(_BF16).T                                    # [D, O]
    # [ds, p, ot, o] -> [ot, p, ds, o] -> [ot*p, ds, o]: 8KB contiguous lines
    Wprep = np.ascontiguousarray(
        WT.reshape(DS, P, NOT, P).transpose(2, 1, 0, 3)
    ).reshape(NOT * P, DS, P)
    bprep = np.ascontiguousarray(b.astype(np.float32).reshape(NOT, P).T)

    in_maps = []
    for c in range(_NCORES):
        t0 = c * _TC
        in_maps.append(
            {
                "xp": np.ascontiguousarray(xprep[:, :, t0 : t0 + _TC]),
                "Wp": Wprep,
                "bvec": bprep,
            }
        )

    trace = os.environ.get("KERNEL_TRACE", "0") == "1"
    res = run_bass_kernel_spmd(
        nc,
        in_maps,
        core_ids=list(range(_NCORES)),
        trace=trace,
    )
    LAST_RESULT = res

    out = np.empty((_T, _O), dtype=np.float32)
    for c, r in enumerate(res.results):
        out[c * _TC : (c + 1) * _TC, :] = r["out"].T
    return out.reshape(_B, _S, _O)


# revision 28
# speedup vs baseline: 1.0133x; 1.0062x over previous
"""LoRA Linear kernel for 8x TRN2 NeuronCores (Bass/Tile).

Computes  y = x @ W^T + b + 2.0 * ((x @ A^T) @ B^T)   for
  x [4, 2048, 4096] f32, W [4096, 4096], b [4096], A [16, 4096], B [4096, 16].

Strategy (v7):
  - Algebraic fold: with dropout=0 the LoRA path is linear, so
    W_eff = W + 2.0 * (B @ A) merged on the host (rank-16 update, 0.1% of
    the problem FLOPs).  The device runs a single dense matmul + bias.
  - Data-parallel over tokens: 8192 tokens -> 1024 per core.
  - Stationary operand is the W o-tile [128d, 128o]; the moving operand
    sweeps tokens, so one Ldweights serves the two 512-token chunks
    (redundant Ldweights are deduped post-tile; they cost ~46ns each).
  - Output computed as [O, TC] per core (o on partitions); host transposes.
    That layout makes the bias a per-partition constant, added for free
    during the PSUM->SBUF drain (ACT activation bias / DVE tensor_scalar).
  - x arrives in ds-major 1MB slices on both HWDGE queues so the first
    o-tile's accumulation chain can start ~12us in, tracking arrivals.
  - W is host-prepacked per o-tile so each DMA line is 8KB contiguous.
"""

import os

import numpy as np
import ml_dtypes

_BF16 = ml_dtypes.bfloat16

# Problem constants (hardcoded per harness contract).
_B, _S, _D, _O, _R = 4, 2048, 4096, 4096, 16
_T = _B * _S          # 8192 tokens
_NCORES = 8
_TC = _T // _NCORES   # 1024 tokens per core

P = 128
DS = _D // P          # 32 contraction subtiles
NOT = _O // P         # 32 o-tiles
TCH = 512             # token chunk (moving N)
NCH = _TC // TCH      # 2 chunks per core

_cache = {}

# Set by kernel() when KERNEL_TRACE=1; read by test.py for exec_time_ns.
LAST_RESULT = None


def _build_module():
    import concourse.bacc as bacc
    import concourse.mybir as mybir
    import concourse.tile as tile
    from concourse.bass import ts

    bf16 = mybir.dt.bfloat16
    f32 = mybir.dt.float32

    nc = bacc.Bacc("TRN2", target_bir_lowering=False, debug=False)
    xp_d = nc.dram_tensor("xp", [P, DS, _TC], bf16, kind="ExternalInput")
    Wp_d = nc.dram_tensor("Wp", [NOT * P, DS, P], bf16, kind="ExternalInput")
    bvec_d = nc.dram_tensor("bvec", [P, NOT], f32, kind="ExternalInput")
    out_d = nc.dram_tensor("out", [_O, _TC], f32, kind="ExternalOutput")

    with tile.TileContext(nc) as tc:
        with (
            tc.tile_pool(name="const", bufs=1) as cpool,
            tc.tile_pool(name="wpool", bufs=6) as wpool,
            tc.tile_pool(name="opool", bufs=3) as opool,
            tc.tile_pool(name="ps_mm", bufs=3, space="PSUM") as ps_pool,
        ):
            NG = 8           # ds-group x tiles; each holds both token chunks
            GD = DS // NG    # 4 ds per group
            xg = [
                cpool.tile([P, GD, _TC], bf16, name=f"xg{g}") for g in range(NG)
            ]
            b_sb = cpool.tile([P, NOT], f32)

            def xmov(ds, c):
                return xg[ds // GD][:, ds % GD, ts(c, TCH)]

            # Head DMA order is the critical path to the first matmul: the
            # chain needs xg0 + W0, so both are split across the two HWDGE
            # queues ahead of everything else (bvec is only needed at the
            # first drain ~35us in; the rest of x streams behind).
            W01 = [
                cpool.tile([P, DS, P], bf16, name=f"W0{ot}") for ot in range(2)
            ]
            H = GD // 2
            nc.sync.dma_start(xg[0][:, 0:H, :], xp_d[:, 0:H, :])
            nc.scalar.dma_start(xg[0][:, H:GD, :], xp_d[:, H:GD, :])
            nc.sync.dma_start(W01[0][:, 0 : DS // 2, :], Wp_d[0:P, 0 : DS // 2, :])
            nc.scalar.dma_start(
                W01[0][:, DS // 2 : DS, :], Wp_d[0:P, DS // 2 : DS, :]
            )
            nc.scalar.dma_start(xg[1][:], xp_d[:, GD : 2 * GD, :])
            nc.sync.dma_start(W01[1][:, 0 : DS // 2, :], Wp_d[P : 2 * P, 0 : DS // 2, :])
            nc.sync.dma_start(b_sb[:], bvec_d[:, :])
            nc.scalar.dma_start(
                W01[1][:, DS // 2 : DS, :], Wp_d[P : 2 * P, DS // 2 : DS, :]
            )
            for g in range(2, NG):
                q = nc.sync if g % 2 == 0 else nc.scalar
                q.dma_start(xg[g][:], xp_d[:, g * GD : (g + 1) * GD, :])

            def drain(ot, ps):
                # Drain PSUM -> SBUF with the bias folded in (per-partition
                # constant in this [o, t] layout); ACT and DVE work
                # different banks in parallel; out DMAs alternate queues.
                QW = TCH // 2
                bias = b_sb[:, ot : ot + 1]
                for c in range(NCH):
                    for h in range(2):
                        q = c * 2 + h
                        qt = opool.tile([P, QW], f32, name=f"ot_q{q}")
                        if h == 0:
                            nc.scalar.add(qt[:], ps[c][:, ts(h, QW)], bias)
                        else:
                            nc.vector.tensor_scalar_add(
                                qt[:], ps[c][:, ts(h, QW)], bias
                            )
                        dq = nc.sync if q % 2 == 0 else nc.scalar
                        dq.dma_start(out_d[ts(ot, P), q * QW : (q + 1) * QW], qt[:])

            # Phase 1: o-tiles 0 and 1 interleaved by ds-group, consuming
            # each arriving x slice twice so the PE keeps pace with the
            # x DMA stream instead of stalling on it.
            ps01 = [
                [ps_pool.tile([P, TCH], f32, name=f"ps{c}") for c in range(NCH)]
                for _ in range(2)
            ]
            for g in range(NG):
                for ot in range(2):
                    for dsl in range(GD):
                        ds = g * GD + dsl
                        for c in range(NCH):
                            nc.tensor.matmul(
                                ps01[ot][c][:],
                                W01[ot][:, ds, :],
                                xmov(ds, c),
                                start=(ds == 0),
                                stop=(ds == DS - 1),
                            )
            for ot in range(2):
                drain(ot, ps01[ot])

            # Phase 2: remaining o-tiles, x fully resident, pure stream.
            for ot in range(2, NOT):
                Wt = wpool.tile([P, DS, P], bf16)
                nc.scalar.dma_start(Wt[:], Wp_d[ts(ot, P), :, :])
                ps = [
                    ps_pool.tile([P, TCH], f32, name=f"ps{c}") for c in range(NCH)
                ]
                for ds in range(DS):
                    for c in range(NCH):
                        nc.tensor.matmul(
                            ps[c][:],
                            Wt[:, ds, :],
                            xmov(ds, c),
                            start=(ds == 0),
                            stop=(ds == DS - 1),
                        )
                drain(ot, ps)

    _dedup_ldweights(nc, mybir)
    nc.compile()
    return nc


def _dedup_ldweights(nc, mybir):
    """Drop PE Ldweights that reload the stationary already in the array.

    The tile pass lowers every matmul to an Ldweights+Matmult pair even when
    consecutive matmuls share the stationary operand.  The redundant reload
    costs PE cycles (~46ns exposed per pair at N=512).  Weights persist in
    the array across Matmults, so a back-to-back identical Ldweights with no
    semaphore activity is dead.
    """
    n_drop = 0
    for fn in nc.m.functions:
        for blk in fn.blocks:
            insts = blk.instructions
            new = []
            prev_key = None
            for inst in insts:
                if inst.engine != mybir.EngineType.PE:
                    new.append(inst)
                    continue
                if isinstance(inst, mybir.InstLdweights):
                    key = str(inst.ins[0])
                    if (
                        key == prev_key
                        and not inst.has_wait()
                        and not inst.has_update()
                    ):
                        n_drop += 1
                        continue
                    prev_key = key
                elif isinstance(inst, mybir.InstMatmult):
                    if inst.is_transpose:
                        prev_key = None
                elif isinstance(inst, mybir.InstEventSemaphore):
                    pass
                else:
                    prev_key = None
                new.append(inst)
            blk.instructions = new
    if os.environ.get("KERNEL_DEBUG"):
        print(f"_dedup_ldweights: dropped {n_drop}")


def kernel(x, W, b, lora_A, lora_B):
    global LAST_RESULT
    from concourse.bass_utils import run_bass_kernel_spmd

    if "nc" not in _cache:
        _cache["nc"] = _build_module()
    nc = _cache["nc"]

    # Fold the LoRA rank-16 update into W (dropout=0 makes it exact):
    # y = x @ (W + 2 B A)^T + b
    Weff = W.astype(np.float64) + 2.0 * (
        lora_B.astype(np.float64) @ lora_A.astype(np.float64)
    )

    xf = np.ascontiguousarray(x.reshape(_T, _D)).astype(_BF16)
    xT = np.ascontiguousarray(xf.T)                              # [D, T]
    # [D, T] -> [p, ds, T] so each DMA line is contiguous per partition
    xprep = np.ascontiguousarray(xT.reshape(DS, P, _T).transpose(1, 0, 2))
    WT = Weff.astype(_BF16).T                                    # [D, O]
    # [ds, p, ot, o] -> [ot, p, ds, o] -> [ot*p, ds, o]: 8KB contiguous lines
    Wprep = np.ascontiguousarray(
        WT.reshape(DS, P, NOT, P).transpose(2, 1, 0, 3)
    ).reshape(NOT * P, DS, P)
    bprep = np.ascontiguousarray(b.astype(np.float32).reshape(NOT, P).T)

    in_maps = []
    for c in range(_NCORES):
        t0 = c * _TC
        in_maps.append(
            {
                "xp": np.ascontiguousarray(xprep[:, :, t0 : t0 + _TC]),
                "Wp": Wprep,
                "bvec": bprep,
            }
        )

    trace = os.environ.get("KERNEL_TRACE", "0") == "1"
    res = run_bass_kernel_spmd(
        nc,
        in_maps,
        core_ids=list(range(_NCORES)),
        trace=trace,
    )
    LAST_RESULT = res

    out = np.empty((_T, _O), dtype=np.float32)
    for c, r in enumerate(res.results):
        out[c * _TC : (c + 1) * _TC, :] = r["out"].T
    return out.reshape(_B, _S, _O)
